# revision 25
# baseline (speedup 1.0000x reference)
"""Trainium2 Bass kernel for nn_Attention_87668872446719.

Patch-attention module: v = Conv3x3(x); xe = PatchEmbed(x); q,k = Linear(xe);
attn = softmax(q k^T / sqrt(hd)); out = Fold(attn @ Unfold(v)); out = Conv1x1(out).

Identity (validated): per channel c with head h = c // 32,
    folded[c, patch n, off o] = sum_m attn[h, n, m] * v[c, m, o]
Sharding: core = (image b, half s); s splits each 16x16 patch into its
top/bottom 8 rows (o = ki*16+kj, ki in [8s, 8s+8)).

v7 design (vs v6 which did c-partitioned conv + two 256B-run DRAM scatters):
  - conv-direct-V: conv computed m-partitioned (lhsT = xcol[:, o, m] chunks,
    rhs = wvT[27, 256]) so V lands in SBUF as vh[m, h, o, cl] with NO DRAM
    round trip (saves 25.7MB of HBM traffic + 50K tiny descriptors)
  - mix is V-stationary: out[(o4, cl32), n] = vh_blk^T @ AT_h, accumulated
    over the two m chunks (128+68); A is normalized BEFORE the PE transpose
    so no separate 1/rowsum fold is needed (rows of A sum to 1, which also
    keeps the v_b-via-obias fold valid)
  - F round trip uses [o, c, n] DRAM layout: writes/reads are 392B runs
    (vs 256B), batched into ~100-400KB dma_starts; 8 octile tensors so proj
    reads can start before the whole mix finishes
  - proj reads [c, (16o, 196n)] tiles, writes out[oc, o, n] with 1.5KB+ runs
  - evictions (PSUM->SBUF, the only engines that can touch PSUM) alternate
    DVE/ACT; conv packs 2 offsets per psum tile to amortize fixed costs
"""
from contextlib import ExitStack

import numpy as np
import ml_dtypes

import concourse.bass as bass
import concourse.tile as tile
from concourse import bacc, mybir
from concourse.bass_utils import run_bass_kernel_spmd

B, CIN, H, W = 4, 3, 224, 224
P = 16
DIM = 256
HEADS = 8
Hp = Wp = 14
N = Hp * Wp            # 196 patches
HD = DIM // HEADS      # 32
KI = 8                 # patch rows per core
OFF = KI * P           # 128 within-patch pixels per core
NPIX = N * OFF         # 25088 pixels per core
MCHUNK = (128, 68)     # m (patch) split for the contraction dim
NOCT = 8               # F octiles (16 offsets each)
BF = mybir.dt.bfloat16
F32 = mybir.dt.float32
AFT = mybir.ActivationFunctionType
AX = mybir.AxisListType.X

_CACHE = {}


def _build():
    nc = bacc.Bacc("TRN2", target_bir_lowering=False, debug=False)

    xcol_d = nc.declare_dram_parameter("xcol", [27, OFF, N], BF, isOutput=False)
    patches_d = nc.declare_dram_parameter("patches", [128, 6, N], BF, isOutput=False)
    pwT_d = nc.declare_dram_parameter("pwT", [128, 6, DIM], BF, isOutput=False)
    qkwT_d = nc.declare_dram_parameter("qkwT", [128, 2, 2 * DIM], BF, isOutput=False)
    wvT_d = nc.declare_dram_parameter("wvT", [27, DIM], BF, isOutput=False)
    projwT_d = nc.declare_dram_parameter("projwT", [128, 2, DIM], BF, isOutput=False)
    pbias_d = nc.declare_dram_parameter("pbias", [128, 2], F32, isOutput=False)
    obias_d = nc.declare_dram_parameter("obias", [128, 2], F32, isOutput=False)
    ident_d = nc.declare_dram_parameter("ident", [128, 128], BF, isOutput=False)
    out_d = nc.declare_dram_parameter("out", [DIM, OFF, N], BF, isOutput=True)

    # F octiles: [16 o, 32 cl, 8 h, 196 n] each — channel order (cl, h) so
    # the mix write merges (head-pair, n) into one 784B run per partition;
    # projwT rows are host-permuted to match.
    fdram = [nc.dram_tensor("fdram%d" % g, [16, HD, HEADS, N], BF)
             for g in range(NOCT)]

    with tile.TileContext(nc) as tc, ExitStack() as ctx:
        const = ctx.enter_context(tc.tile_pool(name="const", bufs=1))
        sb = ctx.enter_context(tc.tile_pool(name="sb", bufs=2))
        # AT is produced in stage C but consumed through all of stage E —
        # keep every instance live.
        atp = ctx.enter_context(tc.tile_pool(name="atp", bufs=16))
        stat = ctx.enter_context(tc.tile_pool(name="stat", bufs=4))
        vhp = ctx.enter_context(tc.tile_pool(name="vhp", bufs=1))
        xcp = ctx.enter_context(tc.tile_pool(name="xcp", bufs=3))
        fsp = ctx.enter_context(tc.tile_pool(name="fsp", bufs=4))
        frp = ctx.enter_context(tc.tile_pool(name="frp", bufs=3))
        osp = ctx.enter_context(tc.tile_pool(name="osp", bufs=4))
        # PSUM: conv [msz,512] 1 bank x2; mix [128,392] 1 bank x3;
        # shared (abc + proj) [128,392] 1 bank x3 -> 8 banks total.
        # (conv runs far ahead of the mix, so 2 bufs suffice there; mix and
        # proj need the runway so the PE doesn't wait on eviction recycle.)
        cps = ctx.enter_context(tc.tile_pool(name="cps", bufs=2, space="PSUM"))
        mps = ctx.enter_context(tc.tile_pool(name="mps", bufs=3, space="PSUM"))
        sps = ctx.enter_context(tc.tile_pool(name="sps", bufs=3, space="PSUM"))

        # ---- constants ----
        def cload(shape, dt, dram, tag):
            t = const.tile(shape, dt, tag=tag, name=tag)
            nc.sync.dma_start(t[:], dram[:])
            return t

        patches_t = cload([128, 6, N], BF, patches_d, "c_patches")
        pwT_t = cload([128, 6, DIM], BF, pwT_d, "c_pwT")
        qkwT_t = cload([128, 2, 2 * DIM], BF, qkwT_d, "c_qkwT")
        wvT_t = cload([27, DIM], BF, wvT_d, "c_wvT")
        projwT_t = cload([128, 2, DIM], BF, projwT_d, "c_projwT")
        pbias_t = cload([128, 2], F32, pbias_d, "c_pbias")
        obias_t = cload([128, 2], F32, obias_d, "c_obias")
        ident_t = cload([128, 128], BF, ident_d, "c_ident")

        # V resident in SBUF as one tile per (m-chunk, o-quad), layout
        # [m, h, o4, cl]: the mix lhsT slice [m, h, :, :] is a contiguous
        # 128-wide block (single free dim, as LDWEIGHTS requires), and
        # per-quad tiles keep conv-write/mix-read dependencies precise.
        vhq = {}

        # ---- stage A: xeT[c, n] = patch embed (transposed) ----
        xeT = []
        for cc in range(2):
            ps = sps.tile([128, N], F32, tag="sp")
            for kc in range(6):
                nc.tensor.matmul(
                    ps[:], pwT_t[:, kc, cc * 128:(cc + 1) * 128],
                    patches_t[:, kc, :], start=(kc == 0), stop=(kc == 5))
            xt = sb.tile([128, N], BF, tag="xeT%d" % cc)
            nc.vector.tensor_scalar_add(xt[:], ps[:], pbias_t[:, cc:cc + 1])
            xeT.append(xt)

        # ---- stage B/C: per-head q/k, scores, softmax (normalized), AT ----
        AT = []     # AT[h][mci] : [msz, N] bf16 (A^T, normalized)
        for h in range(HEADS):
            qT = sb.tile([HD, N], BF, tag="qT")
            kT = sb.tile([HD, N], BF, tag="kT")
            for dst, joff in ((qT, h * HD), (kT, DIM + h * HD)):
                ps = sps.tile([HD, N], F32, tag="sp")
                for cc in range(2):
                    nc.tensor.matmul(
                        ps[:], qkwT_t[:, cc, joff:joff + HD], xeT[cc][:],
                        start=(cc == 0), stop=(cc == 1))
                nc.scalar.copy(dst[:], ps[:])

            Ah = []
            nbase = 0
            for nci, nsz in enumerate(MCHUNK):
                ps = sps.tile([nsz, N], F32, tag="sp")
                nc.tensor.matmul(ps[:], qT[:, nbase:nbase + nsz], kT[:],
                                 start=True, stop=True)
                mx = stat.tile([nsz, 1], F32, tag="mx")
                nc.vector.reduce_max(mx[:], ps[:], axis=AX, negate=True)
                ex = sb.tile([nsz, N], F32, tag="ex")
                nc.scalar.activation(ex[:], ps[:], AFT.Exp, bias=mx[:])
                sm = stat.tile([nsz, 1], F32, tag="smm")
                nc.vector.reduce_sum(sm[:], ex[:], axis=AX)
                rc = stat.tile([nsz, 1], F32, tag="rc")
                nc.vector.reciprocal(rc[:], sm[:])
                ab = sb.tile([nsz, N], BF, tag="ab")
                nc.vector.tensor_scalar_mul(ab[:], ex[:], rc[:])
                Ah.append(ab)
                nbase += nsz

            ATh = []
            mbase = 0
            for mci, msz in enumerate(MCHUNK):
                at = atp.tile([msz, N], BF, tag="at%d" % mci)
                nbase = 0
                for nci, nsz in enumerate(MCHUNK):
                    pt = sps.tile([msz, nsz], BF, tag="sp")
                    nc.tensor.transpose(pt[:], Ah[nci][:, mbase:mbase + msz],
                                        ident_t[:nsz, :nsz])
                    if nci == 0:
                        nc.vector.tensor_copy(at[:, nbase:nbase + nsz], pt[:])
                    else:
                        nc.scalar.copy(at[:, nbase:nbase + nsz], pt[:])
                    nbase += nsz
                ATh.append(at)
                mbase += msz
            AT.append(ATh)

        # ---- stages D/E/F interleaved over offsets ----
        # D: conv-direct per o (2 offsets packed per psum tile):
        #    psum[msz, (2o, 256c)] = xcol[27, o, m]^T @ wvT[27, 256]
        #    evict -> vh[m, h, o, cl]  (c = 32h + cl)
        # E: mix per (b, h): psum[(o4, cl32), 2h x n] -> fdram[o, c, n]
        # F: proj per octile g once fdram[g] complete
        ei = 0  # eviction engine alternator

        def evict_engine():
            nonlocal ei
            ei += 1
            return nc.vector if ei % 2 == 0 else nc.scalar

        xcol_tiles = {}

        def xload(q):
            xt = xcp.tile([27, 4, N], BF, tag="xc")
            nc.sync.dma_start(xt[:], xcol_d[:, 4 * q:4 * q + 4, :])
            xcol_tiles[q] = xt

        def conv_quad(q):
            xt = xcol_tiles.pop(q)
            for mci, msz in enumerate(MCHUNK):
                vhq[(mci, q)] = vhp.tile(
                    [msz, HEADS, 4, HD], BF, tag="vh_%d_%d" % (mci, q),
                    name="vh_%d_%d" % (mci, q))
            # two psum tiles (2 offsets each) per m-chunk
            for oi in range(2):
                for mci, msz in enumerate(MCHUNK):
                    mbase = 128 * mci
                    ps = cps.tile([msz, 512], F32, tag="cp")
                    for orel in range(2):
                        nc.tensor.matmul(
                            ps[:, orel * 256:(orel + 1) * 256],
                            xt[:, 2 * oi + orel, mbase:mbase + msz],
                            wvT_t[:], start=True, stop=True)
                    # evict [msz, (2o, 8h, 32cl)] -> vhq[m, 8h, 2o, 32cl]
                    eng = evict_engine()
                    dst = vhq[(mci, q)][:, :, 2 * oi:2 * oi + 2, :]
                    src = ps[:].rearrange("m (o h c) -> m h o c", o=2, h=HEADS)
                    if eng is nc.vector:
                        nc.vector.tensor_copy(dst, src)
                    else:
                        nc.scalar.copy(dst, src)

        def mix_block(b, hh):
            # hh = head pair index (heads 2hh, 2hh+1); psum [128, 2*196]
            ps = mps.tile([128, 2 * N], F32, tag="mp")
            for hrel in range(2):
                h = 2 * hh + hrel
                for mci, msz in enumerate(MCHUNK):
                    nc.tensor.matmul(
                        ps[:, hrel * N:(hrel + 1) * N],
                        vhq[(mci, b)][:, h, :, :],
                        AT[h][mci][:],
                        start=(mci == 0), stop=(mci == 1))
            fs = fsp.tile([128, 2 * N], BF, tag="fs")
            eng = evict_engine()
            if eng is nc.vector:
                nc.vector.tensor_copy(fs[:], ps[:])
            else:
                nc.scalar.copy(fs[:], ps[:])
            # dst (o4, cl32, (t n) 392) iterates in the same flat order as
            # src partitions (o4, cl32) x free (t, n)
            g, brel = divmod(b, 4)
            fd = fdram[g][4 * brel:4 * brel + 4, :,
                          2 * hh:2 * hh + 2, :].rearrange(
                              "o cl t n -> o cl (t n)")
            nc.sync.dma_start(fd, fs[:])

        fr_tiles = {}

        def fread_oct(g):
            # issue the two F-octile reads early (an octile ahead of proj)
            frs = []
            for cc in range(2):
                fr = frp.tile([128, 16, N], BF, tag="fr%d" % cc,
                              name="fr%d" % cc)
                nc.scalar.dma_start(
                    fr[:], fdram[g].ap().rearrange(
                        "o cl h n -> (cl h) o n")[cc * 128:(cc + 1) * 128])
                frs.append(fr)
            fr_tiles[g] = frs

        def proj_oct(g):
            frs = fr_tiles.pop(g)
            for pc in range(8):   # 8 pixel chunks of (2 o, 196 n)
                ots = osp.tile([128, 2, 2, N], BF, tag="ot")
                for occ in range(2):
                    ps = sps.tile([128, 2 * N], F32, tag="sp")
                    for cc in range(2):
                        nc.tensor.matmul(
                            ps[:], projwT_t[:, cc, occ * 128:(occ + 1) * 128],
                            frs[cc][:, 2 * pc:2 * pc + 2, :].rearrange(
                                "c o n -> c (o n)"),
                            start=(cc == 0), stop=(cc == 1))
                    eng = evict_engine()
                    if eng is nc.vector:
                        nc.vector.tensor_scalar_add(
                            ots[:, occ, :, :],
                            ps[:].rearrange("p (o n) -> p o n", o=2),
                            obias_t[:, occ:occ + 1])
                    else:
                        nc.scalar.activation(
                            ots[:, occ, :, :],
                            ps[:].rearrange("p (o n) -> p o n", o=2),
                            AFT.Identity, bias=obias_t[:, occ:occ + 1])
                o0 = 16 * g + 2 * pc
                nc.gpsimd.dma_start(
                    out_d[:, o0:o0 + 2, :].rearrange(
                        "(t c) o n -> c t o n", t=2),
                    ots[:])

        # software pipelining: conv one quad ahead of the mix consuming it;
        # F-octile reads issued as soon as the octile is written; proj one
        # octile behind so its reads are in flight while the PE mixes.
        NQ = OFF // 4                   # 32 o-quads
        xload(0)
        xload(1)
        conv_quad(0)
        for q in range(NQ):
            if q + 2 < NQ:
                xload(q + 2)
            if q + 1 < NQ:
                conv_quad(q + 1)
            for hh in range(HEADS // 2):
                mix_block(q, hh)
            if q % 4 == 3:
                fread_oct(q // 4)
                if q >= 11:
                    proj_oct(q // 4 - 2)
        proj_oct(NOCT - 2)
        proj_oct(NOCT - 1)

    nc.compile()
    return nc


def _host_prep(inputs):
    """Returns per-core in_maps."""
    x = np.asarray(inputs["x"], np.float32)
    patch_w = np.asarray(inputs["patch_w"], np.float32)
    patch_b = np.asarray(inputs["patch_b"], np.float32)
    qk_w = np.asarray(inputs["qk_w"], np.float32)
    v_w = np.asarray(inputs["v_w"], np.float32)
    v_b = np.asarray(inputs["v_b"], np.float32)
    proj_w = np.asarray(inputs["proj_w"], np.float32).reshape(DIM, DIM)
    proj_b = np.asarray(inputs["proj_b"], np.float32)

    bf = ml_dtypes.bfloat16
    pw = patch_w.reshape(DIM, CIN * P * P)                     # [256, 768]
    pwT = pw.T.reshape(6, 128, DIM).transpose(1, 0, 2)         # [128, 6, 256]
    qkw = qk_w.copy()
    qkw[:DIM] *= HD ** -0.5                                    # fold attn scale
    qkwT = qkw.T.reshape(2, 128, 2 * DIM).transpose(1, 0, 2)   # [128, 2, 512]
    wvT = v_w.reshape(DIM, 27).T                               # [27, 256]
    # proj contraction rows permuted to (cl, h) channel order to match the
    # fdram [o, cl, h, n] layout: row i=(cl*8+h) holds channel c=h*32+cl
    i = np.arange(DIM)
    perm = (i % HEADS) * HD + i // HEADS
    projwT = proj_w.T[perm].reshape(2, 128, DIM).transpose(1, 0, 2)
    pbias = patch_b.reshape(2, 128).T.copy()                   # [128, 2]
    obias = (proj_w @ v_b + proj_b).reshape(2, 128).T.copy()   # [128, 2]

    shared = {
        "pwT": pwT.astype(bf), "qkwT": qkwT.astype(bf),
        "wvT": wvT.astype(bf), "projwT": projwT.astype(bf),
        "pbias": pbias.astype(np.float32), "obias": obias.astype(np.float32),
        "ident": np.eye(128, dtype=bf),
    }

    in_maps = []
    for b in range(B):
        # patches: [768, 196] part order (ci, ki, kj) -> [128, 6, 196]
        p4 = x[b].reshape(CIN, Hp, P, Wp, P).transpose(0, 2, 4, 1, 3)
        patches = p4.reshape(CIN * P * P, N).reshape(6, 128, N)
        patches = patches.transpose(1, 0, 2).astype(bf)
        xpad = np.zeros((CIN, H + 2, W + 2), np.float32)
        xpad[:, 1:-1, 1:-1] = x[b]
        for s in range(2):
            # xcol in (o, m) order: [27, OFF, N]
            cols = np.empty((CIN, 3, 3, KI, P, Hp, Wp), np.float32)
            for dy in range(3):
                for dx in range(3):
                    view = xpad[:, dy:dy + H, dx:dx + W]
                    v4 = view.reshape(CIN, Hp, P, Wp, P)[:, :, 8 * s:8 * s + 8]
                    cols[:, dy, dx] = v4.transpose(0, 2, 4, 1, 3)
            xcol = cols.reshape(27, OFF, N).astype(bf)
            in_maps.append(dict(shared, xcol=xcol, patches=patches))
    return in_maps


def kernel(**inputs):
    if "nc" not in _CACHE:
        _CACHE["nc"] = _build()
    nc = _CACHE["nc"]
    in_maps = _host_prep(inputs)
    res = run_bass_kernel_spmd(nc, in_maps, core_ids=list(range(8)))
    out = np.zeros((B, DIM, H, W), np.float32)
    ov = out.reshape(B, DIM, Hp, P, Wp, P)
    for i, r in enumerate(res.results):
        b, s = divmod(i, 2)
        o = np.asarray(r["out"], dtype=np.float32).reshape(DIM, KI, P, Hp, Wp)
        ov[b, :, :, 8 * s:8 * s + 8, :, :] = o.transpose(0, 3, 1, 4, 2)
    return out


# revision 27
# speedup vs baseline: 1.2331x; 1.2331x over previous
"""Trainium2 Bass kernel for nn_Attention_87668872446719.

Patch-attention module: v = Conv3x3(x); xe = PatchEmbed(x); q,k = Linear(xe);
attn = softmax(q k^T / sqrt(hd)); out = Fold(attn @ Unfold(v)); out = Conv1x1(out).

Identity (validated): per channel c with head h = c // 32,
    folded[c, patch n, off o] = sum_m attn[h, n, m] * v[c, m, o]
Sharding: core = (image b, half s); s splits each 16x16 patch into its
top/bottom 8 rows (o = ki*16+kj, ki in [8s, 8s+8)).

v7 design (vs v6 which did c-partitioned conv + two 256B-run DRAM scatters):
  - conv-direct-V: conv computed m-partitioned (lhsT = xcol[:, o, m] chunks,
    rhs = wvT[27, 256]) so V lands in SBUF as vh[m, h, o, cl] with NO DRAM
    round trip (saves 25.7MB of HBM traffic + 50K tiny descriptors)
  - mix is V-stationary: out[(o4, cl32), n] = vh_blk^T @ AT_h, accumulated
    over the two m chunks (128+68); A is normalized BEFORE the PE transpose
    so no separate 1/rowsum fold is needed (rows of A sum to 1, which also
    keeps the v_b-via-obias fold valid)
  - F round trip uses [o, c, n] DRAM layout: writes/reads are 392B runs
    (vs 256B), batched into ~100-400KB dma_starts; 8 octile tensors so proj
    reads can start before the whole mix finishes
  - proj reads [c, (16o, 196n)] tiles, writes out[oc, o, n] with 1.5KB+ runs
  - evictions (PSUM->SBUF, the only engines that can touch PSUM) alternate
    DVE/ACT; conv packs 2 offsets per psum tile to amortize fixed costs
"""
from contextlib import ExitStack

import numpy as np
import ml_dtypes

import concourse.bass as bass
import concourse.tile as tile
from concourse import bacc, mybir
from concourse.bass_utils import run_bass_kernel_spmd

B, CIN, H, W = 4, 3, 224, 224
P = 16
DIM = 256
HEADS = 8
Hp = Wp = 14
N = Hp * Wp            # 196 patches
HD = DIM // HEADS      # 32
KI = 8                 # patch rows per core
OFF = KI * P           # 128 within-patch pixels per core
NPIX = N * OFF         # 25088 pixels per core
MCHUNK = (128, 68)     # m (patch) split for the contraction dim
NOCT = 8               # F octiles (16 offsets each)
BF = mybir.dt.bfloat16
F32 = mybir.dt.float32
AFT = mybir.ActivationFunctionType
AX = mybir.AxisListType.X

_CACHE = {}


def _build():
    nc = bacc.Bacc("TRN2", target_bir_lowering=False, debug=False)

    xcol_d = nc.declare_dram_parameter("xcol", [27, OFF, N], BF, isOutput=False)
    patches_d = nc.declare_dram_parameter("patches", [128, 6, N], BF, isOutput=False)
    pwT_d = nc.declare_dram_parameter("pwT", [128, 6, DIM], BF, isOutput=False)
    qkwT_d = nc.declare_dram_parameter("qkwT", [128, 2, 2 * DIM], BF, isOutput=False)
    wvT_d = nc.declare_dram_parameter("wvT", [27, DIM], BF, isOutput=False)
    projwT_d = nc.declare_dram_parameter("projwT", [128, 2, DIM], BF, isOutput=False)
    pbias_d = nc.declare_dram_parameter("pbias", [128, 2], F32, isOutput=False)
    obias_d = nc.declare_dram_parameter("obias", [128, 2], F32, isOutput=False)
    ident_d = nc.declare_dram_parameter("ident", [128, 128], BF, isOutput=False)
    out_d = nc.declare_dram_parameter("out", [DIM, OFF, N], BF, isOutput=True)

    # F octiles: [16 o, 32 cl, 8 h, 196 n] each — channel order (cl, h) so
    # the mix write merges (head-pair, n) into one 784B run per partition;
    # projwT rows are host-permuted to match.
    fdram = [nc.dram_tensor("fdram%d" % g, [16, HD, HEADS, N], BF)
             for g in range(NOCT)]

    with tile.TileContext(nc) as tc, ExitStack() as ctx:
        const = ctx.enter_context(tc.tile_pool(name="const", bufs=1))
        sb = ctx.enter_context(tc.tile_pool(name="sb", bufs=2))
        # AT is produced in stage C but consumed through all of stage E —
        # keep every instance live.
        atp = ctx.enter_context(tc.tile_pool(name="atp", bufs=16))
        stat = ctx.enter_context(tc.tile_pool(name="stat", bufs=4))
        vhp = ctx.enter_context(tc.tile_pool(name="vhp", bufs=1))
        xcp = ctx.enter_context(tc.tile_pool(name="xcp", bufs=3))
        fsp = ctx.enter_context(tc.tile_pool(name="fsp", bufs=4))
        frp = ctx.enter_context(tc.tile_pool(name="frp", bufs=3))
        osp = ctx.enter_context(tc.tile_pool(name="osp", bufs=4))
        # PSUM: conv [msz,512] 1 bank x3; mix [128,392] 1 bank x3;
        # shared (abc + proj) [128,392] 1 bank x2 -> 8 banks total
        cps = ctx.enter_context(tc.tile_pool(name="cps", bufs=3, space="PSUM"))
        mps = ctx.enter_context(tc.tile_pool(name="mps", bufs=3, space="PSUM"))
        sps = ctx.enter_context(tc.tile_pool(name="sps", bufs=2, space="PSUM"))

        # ---- constants ----
        def cload(shape, dt, dram, tag):
            t = const.tile(shape, dt, tag=tag, name=tag)
            nc.sync.dma_start(t[:], dram[:])
            return t

        patches_t = cload([128, 6, N], BF, patches_d, "c_patches")
        pwT_t = cload([128, 6, DIM], BF, pwT_d, "c_pwT")
        qkwT_t = cload([128, 2, 2 * DIM], BF, qkwT_d, "c_qkwT")
        wvT_t = cload([27, DIM], BF, wvT_d, "c_wvT")
        projwT_t = cload([128, 2, DIM], BF, projwT_d, "c_projwT")
        pbias_t = cload([128, 2], F32, pbias_d, "c_pbias")
        obias_t = cload([128, 2], F32, obias_d, "c_obias")
        ident_t = cload([128, 128], BF, ident_d, "c_ident")

        # V resident in SBUF as one tile per (m-chunk, o-quad), layout
        # [m, h, o4, cl]: the mix lhsT slice [m, h, :, :] is a contiguous
        # 128-wide block (single free dim, as LDWEIGHTS requires), and
        # per-quad tiles keep conv-write/mix-read dependencies precise.
        vhq = {}

        # ---- stage A: xeT[c, n] = patch embed (transposed) ----
        xeT = []
        for cc in range(2):
            ps = sps.tile([128, N], F32, tag="sp")
            for kc in range(6):
                nc.tensor.matmul(
                    ps[:], pwT_t[:, kc, cc * 128:(cc + 1) * 128],
                    patches_t[:, kc, :], start=(kc == 0), stop=(kc == 5))
            xt = sb.tile([128, N], BF, tag="xeT%d" % cc)
            nc.vector.tensor_scalar_add(xt[:], ps[:], pbias_t[:, cc:cc + 1])
            xeT.append(xt)

        # ---- stage B/C: per-head q/k, scores, softmax (normalized), AT ----
        AT = []     # AT[h][mci] : [msz, N] bf16 (A^T, normalized)
        for h in range(HEADS):
            qT = sb.tile([HD, N], BF, tag="qT")
            kT = sb.tile([HD, N], BF, tag="kT")
            for dst, joff in ((qT, h * HD), (kT, DIM + h * HD)):
                ps = sps.tile([HD, N], F32, tag="sp")
                for cc in range(2):
                    nc.tensor.matmul(
                        ps[:], qkwT_t[:, cc, joff:joff + HD], xeT[cc][:],
                        start=(cc == 0), stop=(cc == 1))
                nc.scalar.copy(dst[:], ps[:])

            Ah = []
            nbase = 0
            for nci, nsz in enumerate(MCHUNK):
                ps = sps.tile([nsz, N], F32, tag="sp")
                nc.tensor.matmul(ps[:], qT[:, nbase:nbase + nsz], kT[:],
                                 start=True, stop=True)
                mx = stat.tile([nsz, 1], F32, tag="mx")
                nc.vector.reduce_max(mx[:], ps[:], axis=AX, negate=True)
                ex = sb.tile([nsz, N], F32, tag="ex")
                nc.scalar.activation(ex[:], ps[:], AFT.Exp, bias=mx[:])
                sm = stat.tile([nsz, 1], F32, tag="smm")
                nc.vector.reduce_sum(sm[:], ex[:], axis=AX)
                rc = stat.tile([nsz, 1], F32, tag="rc")
                nc.vector.reciprocal(rc[:], sm[:])
                ab = sb.tile([nsz, N], BF, tag="ab")
                nc.vector.tensor_scalar_mul(ab[:], ex[:], rc[:])
                Ah.append(ab)
                nbase += nsz

            ATh = []
            mbase = 0
            for mci, msz in enumerate(MCHUNK):
                at = atp.tile([msz, N], BF, tag="at%d" % mci)
                nbase = 0
                for nci, nsz in enumerate(MCHUNK):
                    pt = sps.tile([msz, nsz], BF, tag="sp")
                    nc.tensor.transpose(pt[:], Ah[nci][:, mbase:mbase + msz],
                                        ident_t[:nsz, :nsz])
                    if nci == 0:
                        nc.vector.tensor_copy(at[:, nbase:nbase + nsz], pt[:])
                    else:
                        nc.scalar.copy(at[:, nbase:nbase + nsz], pt[:])
                    nbase += nsz
                ATh.append(at)
                mbase += msz
            AT.append(ATh)

        # ---- stages D/E/F interleaved over offsets ----
        # D: conv-direct per o (2 offsets packed per psum tile):
        #    psum[msz, (2o, 256c)] = xcol[27, o, m]^T @ wvT[27, 256]
        #    evict -> vh[m, h, o, cl]  (c = 32h + cl)
        # E: mix per (b, h): psum[(o4, cl32), 2h x n] -> fdram[o, c, n]
        # F: proj per octile g once fdram[g] complete
        ei = 0  # eviction engine alternator

        def evict_engine():
            nonlocal ei
            ei += 1
            return nc.vector if ei % 2 == 0 else nc.scalar

        xcol_tiles = {}

        def xload(q):
            xt = xcp.tile([27, 4, N], BF, tag="xc")
            nc.sync.dma_start(xt[:], xcol_d[:, 4 * q:4 * q + 4, :])
            xcol_tiles[q] = xt

        def conv_quad(q):
            xt = xcol_tiles.pop(q)
            for mci, msz in enumerate(MCHUNK):
                vhq[(mci, q)] = vhp.tile(
                    [msz, HEADS, 4, HD], BF, tag="vh_%d_%d" % (mci, q),
                    name="vh_%d_%d" % (mci, q))
            # two psum tiles (2 offsets each) per m-chunk
            for oi in range(2):
                for mci, msz in enumerate(MCHUNK):
                    mbase = 128 * mci
                    ps = cps.tile([msz, 512], F32, tag="cp")
                    for orel in range(2):
                        nc.tensor.matmul(
                            ps[:, orel * 256:(orel + 1) * 256],
                            xt[:, 2 * oi + orel, mbase:mbase + msz],
                            wvT_t[:], start=True, stop=True)
                    # evict [msz, (2o, 8h, 32cl)] -> vhq[m, 8h, 2o, 32cl]
                    eng = evict_engine()
                    dst = vhq[(mci, q)][:, :, 2 * oi:2 * oi + 2, :]
                    src = ps[:].rearrange("m (o h c) -> m h o c", o=2, h=HEADS)
                    if eng is nc.vector:
                        nc.vector.tensor_copy(dst, src)
                    else:
                        nc.scalar.copy(dst, src)

        def mix_block(b, hh):
            # hh = head pair index (heads 2hh, 2hh+1); psum [128, 2*196]
            ps = mps.tile([128, 2 * N], F32, tag="mp")
            for hrel in range(2):
                h = 2 * hh + hrel
                for mci, msz in enumerate(MCHUNK):
                    nc.tensor.matmul(
                        ps[:, hrel * N:(hrel + 1) * N],
                        vhq[(mci, b)][:, h, :, :],
                        AT[h][mci][:],
                        start=(mci == 0), stop=(mci == 1))
            fs = fsp.tile([128, 2 * N], BF, tag="fs")
            eng = evict_engine()
            if eng is nc.vector:
                nc.vector.tensor_copy(fs[:], ps[:])
            else:
                nc.scalar.copy(fs[:], ps[:])
            # dst (o4, cl32, (t n) 392) iterates in the same flat order as
            # src partitions (o4, cl32) x free (t, n)
            g, brel = divmod(b, 4)
            fd = fdram[g][4 * brel:4 * brel + 4, :,
                          2 * hh:2 * hh + 2, :].rearrange(
                              "o cl t n -> o cl (t n)")
            nc.sync.dma_start(fd, fs[:])

        fr_tiles = {}

        def fread_oct(g):
            # issue the F-octile reads early (two octiles ahead of proj),
            # split into o-halves spread over the sync and scalar queues
            frs = []
            src = fdram[g].ap().rearrange("o cl h n -> (cl h) o n")
            for cc in range(2):
                fr = frp.tile([128, 16, N], BF, tag="fr%d" % cc,
                              name="fr%d" % cc)
                nc.sync.dma_start(fr[:, 0:8, :],
                                  src[cc * 128:(cc + 1) * 128, 0:8])
                nc.scalar.dma_start(fr[:, 8:16, :],
                                    src[cc * 128:(cc + 1) * 128, 8:16])
                frs.append(fr)
            fr_tiles[g] = frs

        def proj_oct(g):
            frs = fr_tiles.pop(g)
            for pc in range(8):   # 8 pixel chunks of (2 o, 196 n)
                ots = osp.tile([128, 2, 2, N], BF, tag="ot")
                for occ in range(2):
                    ps = sps.tile([128, 2 * N], F32, tag="sp")
                    for cc in range(2):
                        nc.tensor.matmul(
                            ps[:], projwT_t[:, cc, occ * 128:(occ + 1) * 128],
                            frs[cc][:, 2 * pc:2 * pc + 2, :].rearrange(
                                "c o n -> c (o n)"),
                            start=(cc == 0), stop=(cc == 1))
                    eng = evict_engine()
                    if eng is nc.vector:
                        nc.vector.tensor_scalar_add(
                            ots[:, occ, :, :],
                            ps[:].rearrange("p (o n) -> p o n", o=2),
                            obias_t[:, occ:occ + 1])
                    else:
                        nc.scalar.activation(
                            ots[:, occ, :, :],
                            ps[:].rearrange("p (o n) -> p o n", o=2),
                            AFT.Identity, bias=obias_t[:, occ:occ + 1])
                o0 = 16 * g + 2 * pc
                nc.gpsimd.dma_start(
                    out_d[:, o0:o0 + 2, :].rearrange(
                        "(t c) o n -> c t o n", t=2),
                    ots[:])

        # software pipelining: conv one quad ahead of the mix consuming it;
        # F-octile reads issued as soon as the octile is written; proj one
        # octile behind so its reads are in flight while the PE mixes.
        NQ = OFF // 4                   # 32 o-quads
        xload(0)
        xload(1)
        conv_quad(0)
        for q in range(NQ):
            if q + 2 < NQ:
                xload(q + 2)
            if q + 1 < NQ:
                conv_quad(q + 1)
            for hh in range(HEADS // 2):
                mix_block(q, hh)
            if q % 4 == 3:
                fread_oct(q // 4)
                if q >= 11:
                    proj_oct(q // 4 - 2)
        proj_oct(NOCT - 2)
        proj_oct(NOCT - 1)

    nc.compile()
    return nc


def _host_prep(inputs):
    """Returns per-core in_maps."""
    x = np.asarray(inputs["x"], np.float32)
    patch_w = np.asarray(inputs["patch_w"], np.float32)
    patch_b = np.asarray(inputs["patch_b"], np.float32)
    qk_w = np.asarray(inputs["qk_w"], np.float32)
    v_w = np.asarray(inputs["v_w"], np.float32)
    v_b = np.asarray(inputs["v_b"], np.float32)
    proj_w = np.asarray(inputs["proj_w"], np.float32).reshape(DIM, DIM)
    proj_b = np.asarray(inputs["proj_b"], np.float32)

    bf = ml_dtypes.bfloat16
    pw = patch_w.reshape(DIM, CIN * P * P)                     # [256, 768]
    pwT = pw.T.reshape(6, 128, DIM).transpose(1, 0, 2)         # [128, 6, 256]
    qkw = qk_w.copy()
    qkw[:DIM] *= HD ** -0.5                                    # fold attn scale
    qkwT = qkw.T.reshape(2, 128, 2 * DIM).transpose(1, 0, 2)   # [128, 2, 512]
    wvT = v_w.reshape(DIM, 27).T                               # [27, 256]
    # proj contraction rows permuted to (cl, h) channel order to match the
    # fdram [o, cl, h, n] layout: row i=(cl*8+h) holds channel c=h*32+cl
    i = np.arange(DIM)
    perm = (i % HEADS) * HD + i // HEADS
    projwT = proj_w.T[perm].reshape(2, 128, DIM).transpose(1, 0, 2)
    pbias = patch_b.reshape(2, 128).T.copy()                   # [128, 2]
    obias = (proj_w @ v_b + proj_b).reshape(2, 128).T.copy()   # [128, 2]

    shared = {
        "pwT": pwT.astype(bf), "qkwT": qkwT.astype(bf),
        "wvT": wvT.astype(bf), "projwT": projwT.astype(bf),
        "pbias": pbias.astype(np.float32), "obias": obias.astype(np.float32),
        "ident": np.eye(128, dtype=bf),
    }

    in_maps = []
    for b in range(B):
        # patches: [768, 196] part order (ci, ki, kj) -> [128, 6, 196]
        p4 = x[b].reshape(CIN, Hp, P, Wp, P).transpose(0, 2, 4, 1, 3)
        patches = p4.reshape(CIN * P * P, N).reshape(6, 128, N)
        patches = patches.transpose(1, 0, 2).astype(bf)
        xpad = np.zeros((CIN, H + 2, W + 2), np.float32)
        xpad[:, 1:-1, 1:-1] = x[b]
        for s in range(2):
            # xcol in (o, m) order: [27, OFF, N]
            cols = np.empty((CIN, 3, 3, KI, P, Hp, Wp), np.float32)
            for dy in range(3):
                for dx in range(3):
                    view = xpad[:, dy:dy + H, dx:dx + W]
                    v4 = view.reshape(CIN, Hp, P, Wp, P)[:, :, 8 * s:8 * s + 8]
                    cols[:, dy, dx] = v4.transpose(0, 2, 4, 1, 3)
            xcol = cols.reshape(27, OFF, N).astype(bf)
            in_maps.append(dict(shared, xcol=xcol, patches=patches))
    return in_maps


def kernel(**inputs):
    if "nc" not in _CACHE:
        _CACHE["nc"] = _build()
    nc = _CACHE["nc"]
    in_maps = _host_prep(inputs)
    res = run_bass_kernel_spmd(nc, in_maps, core_ids=list(range(8)))
    out = np.zeros((B, DIM, H, W), np.float32)
    ov = out.reshape(B, DIM, Hp, P, Wp, P)
    for i, r in enumerate(res.results):
        b, s = divmod(i, 2)
        o = np.asarray(r["out"], dtype=np.float32).reshape(DIM, KI, P, Hp, Wp)
        ov[b, :, :, 8 * s:8 * s + 8, :, :] = o.transpose(0, 3, 1, 4, 2)
    return out
